# revision 32
# baseline (speedup 1.0000x reference)
"""Trainium2 Bass kernel for the AdaptiveGaussKronrod VJP quadrature problem.

Math (exactly the reference's VJP, with a coarser quadrature partition --
the integrand is analytic and bandlimited, freqs <= 3 rad over [0,1], so
a single 32-point Gauss-Legendre rule reproduces the S=128 GK reference
to ~1e-7 relative; verified on host):

    phi = sin(t (x) freqs)                  [N, D]   N = 32 nodes
    Z   = phi @ W + b                       [N, D]
    G   = (hw)_n * cos(t (x) afreqs) * (1 - tanh(Z)^2)
    out = phi^T @ G                         [D, D]

Sharding: output-column parallel over 8 cores (J = D/8 = 512 columns each).
No collectives; host concatenates the 8 column blocks.

Performance structure (v13, ~39us vs the 45us session baseline; the
8-core-aligned HBM read rate ~250-300 GB/s/core is the front wall):
  - W ships as fp8 e3m4 scaled by 128 (2.1MB/core, clip +-14); the mixed
    bf16 phiT x fp8 W GEMM1 runs at full bf16 rate.  Total rel err
    8.7e-3 vs the 2e-2 gate (bf16-only would be 2.1e-3).
  - phiT/phiN are host-precomputed (N=32 makes them tiny); on-device
    generation cost ~10us of per-op-overhead-bound DVE/ACT work.
  - Each dma_start costs ~650ns of HWDGE sequencer issue time and
    consumers wait on WHOLE-transfer semaphores, so the input stream is
    ordered by consumer and sub-chunked: ebias, phiT, W0 in quarters,
    W1/W2/W3 in halves, hwcn4 + packed phiN between W1 and W2.  All
    DMAs ride the Sync ring (a GpSimd SWDGE out-DMA cost a 4us drain at
    kernel end; Scalar-ring issues fight the ACT table load).
  - phiN is packed 4-quarters-high [128, 1024] (a [32, 4096] transfer
    runs at ~1/4 DMA rate) and doubles as the row-group rhs for a 4-way
    tile_position-packed GEMM2: 4 concurrent MMs in disjoint 32-row
    groups, so GEMM2 costs ~0.5us/group of PE instead of 1.7.
  - Z is replicated to the 4 row offsets by a one-hot REP matmul (only
    the PE can partition-shift); each REP slots into the PE FIFO BEFORE
    the next group's GEMM1 so the epilogue chain (zS cast -> REP ->
    tanh -> Square -> stt, with tanh/Square back-to-back on ACT) hides
    under it.  The x128-scaled bias folds into GEMM1 as a K=1 ones-MM.
  - PSUM evacuation (~680ns per [128,512] f32 copy; PSUM reads are
    2 cycles/elem) alternates DVE/ACT and is the out-phase pacer; out^T
    goes to HBM as bf16 in two 4-tile DMAs per group (host reorders the
    packed i-chunks and upcasts).
  - 16 dummy matmuls at program start warm the HAM clock gate.
"""

import numpy as np

D = 4096
J = D // 8          # output columns per core
P = 128
NQ = 32             # Gauss-Legendre nodes on [0,1]
KT = D // P         # 32 k-tiles over D
G = 4               # j-groups per core
JG = J // G         # 128 columns per group
WSCALE = 128.0

# kept for test.py compatibility (full-resolution reference constants)
_NODES_NEG = np.array([-0.9914553711208126, -0.9491079123427585, -0.8648644233597691,
                       -0.7415311855993945, -0.5860872354676911, -0.4058451513773972,
                       -0.20778495500789848, 0.0])
_WK_HALF = np.array([0.022935322010529224, 0.06309209262997856, 0.10479001032225019,
                     0.14065325971552592, 0.1690047266392679, 0.19035057806478542,
                     0.20443294007529889, 0.20948214108472782])
GK_NODES = np.concatenate([-_NODES_NEG[:-1][::-1], _NODES_NEG])  # [15]
GK_WK = np.concatenate([_WK_HALF[:-1][::-1], _WK_HALF])          # [15]


def _host_constants():
    x, w = np.polynomial.legendre.leggauss(NQ)
    t = (0.5 * (x + 1.0)).astype(np.float32)
    hw = (0.5 * w).astype(np.float32)
    return t, hw


def _patch_act_tables():
    """Force Sin AND Tanh to resolve to one table set so the act-table-load
    pass emits a single load instead of thrashing between sets."""
    import concourse.bacc as bacc_mod
    from concourse import mybir

    if getattr(bacc_mod, "_act_tables_pinned", False):
        return
    orig = bacc_mod.get_activation_tables
    Sin = mybir.ActivationFunctionType.Sin
    Tanh = mybir.ActivationFunctionType.Tanh
    Square = mybir.ActivationFunctionType.Square

    def patched(arch):
        tabs = orig(arch)
        out = {}
        for name, funcs in tabs.items():
            if (Sin in funcs) and (Tanh in funcs):
                out[name] = funcs
            else:
                out[name] = funcs - {Sin, Tanh}
        return out

    bacc_mod.get_activation_tables = patched
    bacc_mod._act_tables_pinned = True


def build_bass():
    """Build and compile the per-core Bass graph (identical on all 8 cores)."""
    from contextlib import ExitStack

    import concourse.bass as bass
    import concourse.tile as tile
    from concourse import bacc, mybir

    _patch_act_tables()

    f32 = mybir.dt.float32
    bf16 = mybir.dt.bfloat16
    fp8 = mybir.dt.float8e3
    Sin = mybir.ActivationFunctionType.Sin
    Tanh = mybir.ActivationFunctionType.Tanh
    Square = mybir.ActivationFunctionType.Square
    Alu = mybir.AluOpType

    nc = bacc.Bacc("TRN2", target_bir_lowering=False, debug=False,
                   enable_asserts=False)

    # w packed (g, k)-tile-major: w[p, (g*KT + k)*JG + jj] =
    #   clip(128*W[128k + p, cols[g*JG + jj]])  as fp8 e3m4
    w_ext = nc.dram_tensor("w", [P, G * KT * JG], fp8, kind="ExternalInput")
    # phiT[p, k*NQ + n] = sin(t_n * freqs[k*128 + p]) (host-precomputed;
    # 0.26MB beats ~10us of per-op-overhead-bound DVE muls + ACT sins)
    phit_ext = nc.dram_tensor("phit", [P, KT * NQ], bf16, kind="ExternalInput")
    # ebias: [brow(512, row 0; 128*b) | REP(128, one-hot)] -- tiny, lands
    # first (gates GEMM1's bias MM and the REP matmuls)
    bpack_ext = nc.dram_tensor("bpack", [NQ, J + P], bf16,
                               kind="ExternalInput")
    # hwcn4 row-replicated x4 (consumed by the G stt after tanh)
    hwcn_ext = nc.dram_tensor("hwcn", [P, J], bf16, kind="ExternalInput")
    # phiN packed 4-quarters-high: phinp[32q + n, c] = sin(t_n * f[1024q+c])
    phinp_ext = nc.dram_tensor("phinp", [P, D // 4], bf16,
                               kind="ExternalInput")
    # out^T packed (g, m)-tile-major, m = GEMM2 issue index (q = m%4, h = m//4,
    # i-chunk = 2q + h):  out_ext[p, (g*8 + m)*512 + ii] =
    #   out[(2*(m%4) + m//4)*512 + ii, cols[g*JG + p]]
    out_ext = nc.dram_tensor("out", [P, G * 8 * 512], bf16, kind="ExternalOutput")

    with tile.TileContext(nc) as tc, ExitStack() as ctx:
        consts = ctx.enter_context(tc.tile_pool(name="consts", bufs=1))
        wp = ctx.enter_context(tc.tile_pool(name="wp", bufs=1))
        argsp = ctx.enter_context(tc.tile_pool(name="args", bufs=1))
        phip = ctx.enter_context(tc.tile_pool(name="phi", bufs=1))
        work = ctx.enter_context(tc.tile_pool(name="work", bufs=1))
        ostage = ctx.enter_context(tc.tile_pool(name="ostage", bufs=8))
        zps = ctx.enter_context(
            tc.tile_pool(name="zpsum", bufs=2, space=bass.MemorySpace.PSUM))
        g4ps = ctx.enter_context(
            tc.tile_pool(name="g4psum", bufs=1, space=bass.MemorySpace.PSUM))
        ops = ctx.enter_context(
            tc.tile_pool(name="opsum", bufs=5, space=bass.MemorySpace.PSUM))

        # ---- DMA issues (each costs ~600ns of sequencer time) ----
        # Sync ring: cpack first (unblocks args/phiT), then W chunks with
        # W group 0 split in halves for an earlier GEMM1 start, then
        # (later) all out-DMAs.  GpSimd SWDGE: bpack, phiNP.  Scalar ring
        # stays clear for the ACT table load + Sin chain.
        dummy = consts.tile([P, 256], bf16, tag="dummy")
        nc.vector.memset(dummy[:], 0.0)
        phiT = phip.tile([P, KT * NQ], bf16, name="phiT")
        nc.sync.dma_start(phiT[:], phit_ext[:])
        wt = []
        for g in range(G):
            w_sb = wp.tile([P, KT * JG], fp8, tag=f"wt{g}", name=f"wt{g}")
            wt.append(w_sb)
        # epack right after phiT (it gates epilogue_0); W0 in quarters and
        # W1 in halves so GEMM1 chases the stream at sub-chunk granularity;
        # phiN slots between W1 and W2 (needed at GEMM2_0)
        bpk = consts.tile([NQ, J + P], bf16, tag="bpack")
        nc.sync.dma_start(bpk[:], bpack_ext[:])
        brow = bpk[0:1, 0:J]
        REP = bpk[:, J:J + P]
        KJ = KT * JG
        for c in range(4):
            nc.sync.dma_start(wt[0][:, c * KJ // 4:(c + 1) * KJ // 4],
                              w_ext[:, c * KJ // 4:(c + 1) * KJ // 4])
        for c in range(2):
            nc.sync.dma_start(wt[1][:, c * KJ // 2:(c + 1) * KJ // 2],
                              w_ext[:, KJ + c * KJ // 2:KJ + (c + 1) * KJ // 2])
        hwcn4 = consts.tile([P, J], bf16, tag="hwcn4")
        nc.sync.dma_start(hwcn4[:], hwcn_ext[:])
        phiNP = consts.tile([P, D // 4], bf16, tag="phiNP")
        nc.sync.dma_start(phiNP[:], phinp_ext[:])
        for c in range(2):
            nc.sync.dma_start(wt[2][:, c * KJ // 2:(c + 1) * KJ // 2],
                              w_ext[:, 2 * KJ + c * KJ // 2:2 * KJ + (c + 1) * KJ // 2])
        for c in range(2):
            nc.sync.dma_start(wt[3][:, c * KJ // 2:(c + 1) * KJ // 2],
                              w_ext[:, 3 * KJ + c * KJ // 2:3 * KJ + (c + 1) * KJ // 2])

        # PE warm-up: dummies dispatch right after the start barrier (the
        # memset is the first DVE op) so the HAM clock gate flips to K=8/8
        # during GEMM1_0
        wps = ops.tile([P, 512], f32, tag="opsum", name="warmps")
        for i in range(24):
            nc.tensor.matmul(wps[:, 0:128], lhsT=dummy[:, 0:128],
                             rhs=dummy[:, 128:256], start=True, stop=True)

        zero_c = consts.tile([P, 1], f32, tag="zero_c")
        nc.vector.memset(zero_c[:], 0.0)
        ones_c = consts.tile([1, P], bf16, tag="ones_c")
        nc.vector.memset(ones_c[:], 1.0)

        # first ScalarE op: pulls the ACT table load to kernel start
        scratch = consts.tile([P, 1], f32, tag="scratch")
        nc.scalar.activation(scratch[:], zero_c[:], Tanh, bias=zero_c[:])

        # ---- per-group pipeline ----
        zS = work.tile([NQ, J], bf16, tag="zS")
        y4 = work.tile([P, J], f32, tag="y4")
        s4 = work.tile([P, J], f32, tag="s4")
        g4 = work.tile([P, J], bf16, tag="g4")
        z_t = [None] * G

        def gemm1(g):
            # the K=1 ones-matmul folds in the (x128-scaled) bias
            z = zps.tile([NQ, JG], f32, tag="zpsum", name=f"z{g}")
            z_t[g] = z
            nc.tensor.matmul(z[:], lhsT=ones_c[:, 0:NQ],
                             rhs=brow[:, g * JG:(g + 1) * JG],
                             start=True, stop=False)
            for k in range(KT):
                nc.tensor.matmul(z[:], lhsT=phiT[:, k * NQ:(k + 1) * NQ],
                                 rhs=wt[g][:, k * JG:(k + 1) * JG],
                                 start=False, stop=(k == KT - 1))

        z4_t = [None] * G

        def epilogue_pre(g):
            # zS = z/128 as bf16 (bias already folded in via the ones-MM);
            # the REP matmul replicates to the 4 row offsets (PE is the
            # only engine that can partition-shift).  Emitted BEFORE the
            # next group's GEMM1 so it doesn't queue behind it in the PE
            # FIFO.
            sl = slice(g * JG, (g + 1) * JG)
            nc.vector.tensor_scalar_mul(zS[:, sl], z_t[g][:], 1.0 / WSCALE)
            z4p = g4ps.tile([P, JG], f32, tag="g4psum", name=f"z4p{g}")
            z4_t[g] = z4p
            nc.tensor.matmul(z4p[:], lhsT=REP, rhs=zS[:, sl],
                             start=True, stop=True)

        def epilogue(g):
            # tanh/Square back-to-back on ACT, one DVE stt finishes G
            sl = slice(g * JG, (g + 1) * JG)
            nc.scalar.activation(y4[:, sl], z4_t[g][:], Tanh, bias=zero_c[:])
            # Square rides the otherwise-idle GpSimd (plain tensor_tensor,
            # which Pool supports), off the staging-critical DVE/ACT budget
            nc.gpsimd.tensor_mul(s4[:, sl], y4[:, sl], y4[:, sl])
            nc.vector.scalar_tensor_tensor(g4[:, sl], s4[:, sl], 1.0,
                                           hwcn4[:, sl], Alu.subtract,
                                           Alu.mult)

        def gemm2(g):
            sl = slice(g * JG, (g + 1) * JG)
            # 4-way row-packed GEMM2: issue index m -> (q = m%4, h = m//4),
            # i-chunk = 2q + h; host reorders
            ost = ostage.tile([P, 8 * 512], bf16, tag="ostage",
                              name=f"ost{g}")
            # group 0 ships (2,3,3) so the out-stream starts earlier;
            # later groups stay coarse (trailing small DMAs serialize
            # their ~650ns issues at kernel end)
            bounds = (2, 5, 8) if g == 0 else (4, 8)
            lo = 0
            for m in range(8):
                q, h = m % 4, m // 4
                op = ops.tile([P, 512], f32, tag="opsum", name=f"op{g}_{m}")
                nc.tensor.matmul(
                    op[:], lhsT=g4[32 * q:32 * (q + 1), sl],
                    rhs=phiNP[32 * q:32 * (q + 1), h * 512:(h + 1) * 512],
                    tile_position=(32 * q, 0), start=True, stop=True)
                dst = ost[:, m * 512:(m + 1) * 512]
                if m % 2 == 1:
                    nc.scalar.copy(dst, op[:])
                else:
                    nc.vector.tensor_copy(dst, op[:])
                if m + 1 in bounds:
                    nc.sync.dma_start(
                        out_ext[:, (g * 8 + lo) * 512:(g * 8 + m + 1) * 512],
                        ost[:, lo * 512:(m + 1) * 512])
                    lo = m + 1

        # PE order: G1_0, REP_0, G1_1, G2_0, REP_1, G1_2, G2_1, REP_2,
        # G1_3, G2_2, REP_3, G2_3 -- each REP matmul slots before the next
        # GEMM1 so the epilogue chain overlaps it, while the PE stays
        # busy on the next group's GEMM1 during each chain
        gemm1(0)
        epilogue_pre(0)
        gemm1(1)
        epilogue(0)
        gemm2(0)
        epilogue_pre(1)
        gemm1(2)
        epilogue(1)
        gemm2(1)
        epilogue_pre(2)
        gemm1(3)
        epilogue(2)
        gemm2(2)
        epilogue_pre(3)
        epilogue(3)
        gemm2(3)

    nc.compile()
    return nc


_CACHE = {}


def _get_nc():
    if "nc" not in _CACHE:
        _CACHE["nc"] = build_bass()
    return _CACHE["nc"]


def _host_inputs(W, b, freqs, afreqs):
    """Build the shared + per-core input arrays."""
    import ml_dtypes
    bf16 = ml_dtypes.bfloat16
    fp8 = ml_dtypes.float8_e3m4

    t, hw = _host_constants()

    phin = np.sin(np.outer(t, freqs)).astype(np.float32)     # [32, 4096]
    phinp = np.ascontiguousarray(
        phin.reshape(NQ, 4, D // 4).transpose(1, 0, 2).reshape(P, D // 4)
    ).astype(bf16)
    phit = np.ascontiguousarray(
        phin.T.reshape(KT, P, NQ).transpose(1, 0, 2).reshape(P, KT * NQ)
    ).astype(bf16)
    shared = {"phit": phit, "phinp": phinp}
    in_maps = []
    for i in range(8):
        sl = slice(i * J, (i + 1) * J)
        Wc = np.clip(W[:, sl] * WSCALE, -14.0, 14.0).astype(fp8)
        wpack = np.ascontiguousarray(
            Wc.reshape(KT, P, G, JG).transpose(1, 2, 0, 3).reshape(P, G * KT * JG))
        bpack = np.zeros((NQ, J + P), np.float32)
        bpack[0, 0:J] = b[sl] * WSCALE
        bpack[np.arange(P) % NQ, J + np.arange(P)] = 1.0
        hwcn = -hw[:, None] * np.cos(np.outer(t, afreqs[sl]))
        m = dict(shared)
        m["w"] = wpack
        m["bpack"] = bpack.astype(bf16)
        m["hwcn"] = np.ascontiguousarray(np.tile(hwcn, (4, 1))).astype(bf16)
        in_maps.append(m)
    return in_maps


# i-chunk for GEMM2 issue index m: q = m%4 (row group / i-quarter)
_IC_OF_M = np.array([2 * (m % 4) + m // 4 for m in range(8)])


def _unpack_out(res_i):
    """[P, (g*8 + m)*512 + ii] packed out^T -> [D, J] float32."""
    x = res_i.reshape(P, G, 8, 512)        # [p, g, m, ii]
    x = x[:, :, np.argsort(_IC_OF_M), :]   # [p, g, ic, ii]
    outT = x.transpose(1, 0, 2, 3).reshape(J, D)   # [j, i]
    return np.ascontiguousarray(outT.T).astype(np.float32)


def kernel(W, b, freqs, afreqs):
    from concourse.bass_utils import run_bass_kernel_spmd

    W = np.asarray(W, dtype=np.float32)
    b = np.asarray(b, dtype=np.float32)
    freqs = np.asarray(freqs, dtype=np.float32)
    afreqs = np.asarray(afreqs, dtype=np.float32)

    nc = _get_nc()
    in_maps = _host_inputs(W, b, freqs, afreqs)
    res = run_bass_kernel_spmd(nc, in_maps, core_ids=list(range(8)))
    return np.concatenate(
        [_unpack_out(np.asarray(res.results[i]["out"])) for i in range(8)],
        axis=1)


# revision 34
# speedup vs baseline: 1.0330x; 1.0330x over previous
"""Trainium2 Bass kernel for the AdaptiveGaussKronrod VJP quadrature problem.

Math (exactly the reference's VJP, with a coarser quadrature partition --
the integrand is analytic and bandlimited, freqs <= 3 rad over [0,1], so
a single 32-point Gauss-Legendre rule reproduces the S=128 GK reference
to ~1e-7 relative; verified on host):

    phi = sin(t (x) freqs)                  [N, D]   N = 32 nodes
    Z   = phi @ W + b                       [N, D]
    G   = (hw)_n * cos(t (x) afreqs) * (1 - tanh(Z)^2)
    out = phi^T @ G                         [D, D]

Sharding: output-column parallel over 8 cores (J = D/8 = 512 columns each).
No collectives; host concatenates the 8 column blocks.

Performance structure (~38.5us vs the 45us session baseline; the
8-core-aligned HBM read rate ~250-300 GB/s/core is the front wall):
  - W ships as fp8 e3m4 scaled by 128 (2.1MB/core, clip +-14); the mixed
    bf16 phiT x fp8 W GEMM1 runs at full bf16 rate.  Total rel err
    8.7e-3 vs the 2e-2 gate (bf16-only would be 2.1e-3).
  - phiT/phiN are host-precomputed (N=32 makes them tiny); on-device
    generation cost ~10us of per-op-overhead-bound DVE/ACT work.
  - Each dma_start costs ~650ns of HWDGE sequencer issue time and
    consumers wait on WHOLE-transfer semaphores, so the input stream is
    ordered by consumer and sub-chunked: ebias, phiT, W0 in quarters,
    W1/W2/W3 in halves, hwcn4 + packed phiN between W1 and W2.  All
    DMAs ride the Sync ring (a GpSimd SWDGE out-DMA cost a 4us drain at
    kernel end; Scalar-ring issues fight the ACT table load).
  - phiN is packed 4-quarters-high [128, 1024] (a [32, 4096] transfer
    runs at ~1/4 DMA rate) and doubles as the row-group rhs for a 4-way
    tile_position-packed GEMM2: 4 concurrent MMs in disjoint 32-row
    groups, so GEMM2 costs ~0.5us/group of PE instead of 1.7.
  - Z is replicated to the 4 row offsets by a one-hot REP matmul (only
    the PE can partition-shift); each REP slots into the PE FIFO BEFORE
    the next group's GEMM1 so the epilogue chain (zS cast -> REP ->
    tanh -> square -> stt) hides under it.  The square runs on the
    otherwise-idle GpSimd, off the staging-critical DVE/ACT budget.
    The x128-scaled bias folds into GEMM1 as a K=1 ones-MM.
  - PSUM evacuation (~680ns per [128,512] f32 copy; PSUM reads are
    2 cycles/elem) alternates DVE/ACT and is the out-phase pacer; out^T
    goes to HBM as bf16 in two 4-tile DMAs per group (host reorders the
    packed i-chunks and upcasts); group 0 ships (2,3,3) tiles so the
    out-stream starts during the in-stream, later groups stay coarse
    (trailing small DMAs serialize their issue costs at kernel end).
  - 24 dummy matmuls at program start warm the HAM clock gate.
"""

import numpy as np

D = 4096
J = D // 8          # output columns per core
P = 128
NQ = 32             # Gauss-Legendre nodes on [0,1]
KT = D // P         # 32 k-tiles over D
G = 4               # j-groups per core
JG = J // G         # 128 columns per group
WSCALE = 128.0

# kept for test.py compatibility (full-resolution reference constants)
_NODES_NEG = np.array([-0.9914553711208126, -0.9491079123427585, -0.8648644233597691,
                       -0.7415311855993945, -0.5860872354676911, -0.4058451513773972,
                       -0.20778495500789848, 0.0])
_WK_HALF = np.array([0.022935322010529224, 0.06309209262997856, 0.10479001032225019,
                     0.14065325971552592, 0.1690047266392679, 0.19035057806478542,
                     0.20443294007529889, 0.20948214108472782])
GK_NODES = np.concatenate([-_NODES_NEG[:-1][::-1], _NODES_NEG])  # [15]
GK_WK = np.concatenate([_WK_HALF[:-1][::-1], _WK_HALF])          # [15]


def _host_constants():
    x, w = np.polynomial.legendre.leggauss(NQ)
    t = (0.5 * (x + 1.0)).astype(np.float32)
    hw = (0.5 * w).astype(np.float32)
    return t, hw


def _patch_act_tables():
    """Force Sin AND Tanh to resolve to one table set so the act-table-load
    pass emits a single load instead of thrashing between sets."""
    import concourse.bacc as bacc_mod
    from concourse import mybir

    if getattr(bacc_mod, "_act_tables_pinned", False):
        return
    orig = bacc_mod.get_activation_tables
    Sin = mybir.ActivationFunctionType.Sin
    Tanh = mybir.ActivationFunctionType.Tanh
    Square = mybir.ActivationFunctionType.Square

    def patched(arch):
        tabs = orig(arch)
        out = {}
        for name, funcs in tabs.items():
            if (Sin in funcs) and (Tanh in funcs):
                out[name] = funcs
            else:
                out[name] = funcs - {Sin, Tanh}
        return out

    bacc_mod.get_activation_tables = patched
    bacc_mod._act_tables_pinned = True


def build_bass():
    """Build and compile the per-core Bass graph (identical on all 8 cores)."""
    from contextlib import ExitStack

    import concourse.bass as bass
    import concourse.tile as tile
    from concourse import bacc, mybir

    _patch_act_tables()

    f32 = mybir.dt.float32
    bf16 = mybir.dt.bfloat16
    fp8 = mybir.dt.float8e3
    Sin = mybir.ActivationFunctionType.Sin
    Tanh = mybir.ActivationFunctionType.Tanh
    Square = mybir.ActivationFunctionType.Square
    Alu = mybir.AluOpType

    nc = bacc.Bacc("TRN2", target_bir_lowering=False, debug=False,
                   enable_asserts=False)

    # w packed (g, k)-tile-major: w[p, (g*KT + k)*JG + jj] =
    #   clip(128*W[128k + p, cols[g*JG + jj]])  as fp8 e3m4
    w_ext = nc.dram_tensor("w", [P, G * KT * JG], fp8, kind="ExternalInput")
    # phiT[p, k*NQ + n] = sin(t_n * freqs[k*128 + p]) (host-precomputed;
    # 0.26MB beats ~10us of per-op-overhead-bound DVE muls + ACT sins)
    phit_ext = nc.dram_tensor("phit", [P, KT * NQ], fp8, kind="ExternalInput")
    # ebias: [brow(512, row 0; 128*b) | REP(128, one-hot)] -- tiny, lands
    # first (gates GEMM1's bias MM and the REP matmuls)
    bpack_ext = nc.dram_tensor("bpack", [NQ, J + P], bf16,
                               kind="ExternalInput")
    # hwcn4 row-replicated x4 (consumed by the G stt after tanh)
    hwcn_ext = nc.dram_tensor("hwcn", [P, J], bf16, kind="ExternalInput")
    # phiN packed 4-quarters-high: phinp[32q + n, c] = sin(t_n * f[1024q+c])
    phinp_ext = nc.dram_tensor("phinp", [P, D // 4], bf16,
                               kind="ExternalInput")
    # out^T packed (g, m)-tile-major, m = GEMM2 issue index (q = m%4, h = m//4,
    # i-chunk = 2q + h):  out_ext[p, (g*8 + m)*512 + ii] =
    #   out[(2*(m%4) + m//4)*512 + ii, cols[g*JG + p]]
    out_ext = nc.dram_tensor("out", [P, G * 8 * 512], bf16, kind="ExternalOutput")

    with tile.TileContext(nc) as tc, ExitStack() as ctx:
        consts = ctx.enter_context(tc.tile_pool(name="consts", bufs=1))
        wp = ctx.enter_context(tc.tile_pool(name="wp", bufs=1))
        argsp = ctx.enter_context(tc.tile_pool(name="args", bufs=1))
        phip = ctx.enter_context(tc.tile_pool(name="phi", bufs=1))
        work = ctx.enter_context(tc.tile_pool(name="work", bufs=1))
        ostage = ctx.enter_context(tc.tile_pool(name="ostage", bufs=8))
        zps = ctx.enter_context(
            tc.tile_pool(name="zpsum", bufs=2, space=bass.MemorySpace.PSUM))
        g4ps = ctx.enter_context(
            tc.tile_pool(name="g4psum", bufs=1, space=bass.MemorySpace.PSUM))
        ops = ctx.enter_context(
            tc.tile_pool(name="opsum", bufs=5, space=bass.MemorySpace.PSUM))

        # ---- DMA issues (each costs ~600ns of sequencer time) ----
        # Sync ring: cpack first (unblocks args/phiT), then W chunks with
        # W group 0 split in halves for an earlier GEMM1 start, then
        # (later) all out-DMAs.  GpSimd SWDGE: bpack, phiNP.  Scalar ring
        # stays clear for the ACT table load + Sin chain.
        dummy = consts.tile([P, 256], bf16, tag="dummy")
        nc.vector.memset(dummy[:], 0.0)
        phiT = phip.tile([P, KT * NQ], fp8, name="phiT")
        nc.sync.dma_start(phiT[:], phit_ext[:])
        wt = []
        for g in range(G):
            w_sb = wp.tile([P, KT * JG], fp8, tag=f"wt{g}", name=f"wt{g}")
            wt.append(w_sb)
        # epack right after phiT (it gates epilogue_0); W0 in quarters and
        # W1 in halves so GEMM1 chases the stream at sub-chunk granularity;
        # phiN slots between W1 and W2 (needed at GEMM2_0)
        bpk = consts.tile([NQ, J + P], bf16, tag="bpack")
        nc.sync.dma_start(bpk[:], bpack_ext[:])
        brow = bpk[0:1, 0:J]
        REP = bpk[:, J:J + P]
        KJ = KT * JG
        for c in range(4):
            nc.sync.dma_start(wt[0][:, c * KJ // 4:(c + 1) * KJ // 4],
                              w_ext[:, c * KJ // 4:(c + 1) * KJ // 4])
        for c in range(2):
            nc.sync.dma_start(wt[1][:, c * KJ // 2:(c + 1) * KJ // 2],
                              w_ext[:, KJ + c * KJ // 2:KJ + (c + 1) * KJ // 2])
        hwcn4 = consts.tile([P, J], bf16, tag="hwcn4")
        nc.sync.dma_start(hwcn4[:], hwcn_ext[:])
        phiNP = consts.tile([P, D // 4], bf16, tag="phiNP")
        nc.sync.dma_start(phiNP[:], phinp_ext[:])
        for c in range(2):
            nc.sync.dma_start(wt[2][:, c * KJ // 2:(c + 1) * KJ // 2],
                              w_ext[:, 2 * KJ + c * KJ // 2:2 * KJ + (c + 1) * KJ // 2])
        for c in range(2):
            nc.sync.dma_start(wt[3][:, c * KJ // 2:(c + 1) * KJ // 2],
                              w_ext[:, 3 * KJ + c * KJ // 2:3 * KJ + (c + 1) * KJ // 2])

        # PE warm-up: dummies dispatch right after the start barrier (the
        # memset is the first DVE op) so the HAM clock gate flips to K=8/8
        # during GEMM1_0
        wps = ops.tile([P, 512], f32, tag="opsum", name="warmps")
        for i in range(24):
            nc.tensor.matmul(wps[:, 0:128], lhsT=dummy[:, 0:128],
                             rhs=dummy[:, 128:256], start=True, stop=True)

        zero_c = consts.tile([P, 1], f32, tag="zero_c")
        nc.vector.memset(zero_c[:], 0.0)
        ones_c = consts.tile([1, P], bf16, tag="ones_c")
        nc.vector.memset(ones_c[:], 1.0)

        # first ScalarE op: pulls the ACT table load to kernel start
        scratch = consts.tile([P, 1], f32, tag="scratch")
        nc.scalar.activation(scratch[:], zero_c[:], Tanh, bias=zero_c[:])

        # ---- per-group pipeline ----
        zS = work.tile([NQ, J], bf16, tag="zS")
        y4 = work.tile([P, J], f32, tag="y4")
        s4 = work.tile([P, J], f32, tag="s4")
        g4 = work.tile([P, J], bf16, tag="g4")
        z_t = [None] * G

        def gemm1(g):
            # the K=1 ones-matmul folds in the (x128-scaled) bias
            z = zps.tile([NQ, JG], f32, tag="zpsum", name=f"z{g}")
            z_t[g] = z
            nc.tensor.matmul(z[:], lhsT=ones_c[:, 0:NQ],
                             rhs=brow[:, g * JG:(g + 1) * JG],
                             start=True, stop=False)
            for k in range(KT):
                nc.tensor.matmul(z[:], lhsT=phiT[:, k * NQ:(k + 1) * NQ],
                                 rhs=wt[g][:, k * JG:(k + 1) * JG],
                                 start=False, stop=(k == KT - 1))

        z4_t = [None] * G

        def epilogue_pre(g):
            # zS = z/128 as bf16 (bias already folded in via the ones-MM);
            # the REP matmul replicates to the 4 row offsets (PE is the
            # only engine that can partition-shift).  Emitted BEFORE the
            # next group's GEMM1 so it doesn't queue behind it in the PE
            # FIFO.
            sl = slice(g * JG, (g + 1) * JG)
            nc.vector.tensor_scalar_mul(zS[:, sl], z_t[g][:], 1.0 / (WSCALE * 8.0))
            z4p = g4ps.tile([P, JG], f32, tag="g4psum", name=f"z4p{g}")
            z4_t[g] = z4p
            nc.tensor.matmul(z4p[:], lhsT=REP, rhs=zS[:, sl],
                             start=True, stop=True)

        def epilogue(g):
            # tanh/Square back-to-back on ACT, one DVE stt finishes G
            sl = slice(g * JG, (g + 1) * JG)
            nc.scalar.activation(y4[:, sl], z4_t[g][:], Tanh, bias=zero_c[:])
            # Square rides the otherwise-idle GpSimd (plain tensor_tensor,
            # which Pool supports), off the staging-critical DVE/ACT budget
            nc.gpsimd.tensor_mul(s4[:, sl], y4[:, sl], y4[:, sl])
            nc.vector.scalar_tensor_tensor(g4[:, sl], s4[:, sl], 1.0,
                                           hwcn4[:, sl], Alu.subtract,
                                           Alu.mult)

        def gemm2(g):
            sl = slice(g * JG, (g + 1) * JG)
            # 4-way row-packed GEMM2: issue index m -> (q = m%4, h = m//4),
            # i-chunk = 2q + h; host reorders
            ost = ostage.tile([P, 8 * 512], bf16, tag="ostage",
                              name=f"ost{g}")
            # group 0 ships (2,3,3) so the out-stream starts earlier;
            # later groups stay coarse (trailing small DMAs serialize
            # their ~650ns issues at kernel end)
            bounds = (2, 5, 8) if g == 0 else (4, 8)
            lo = 0
            for m in range(8):
                q, h = m % 4, m // 4
                op = ops.tile([P, 512], f32, tag="opsum", name=f"op{g}_{m}")
                nc.tensor.matmul(
                    op[:], lhsT=g4[32 * q:32 * (q + 1), sl],
                    rhs=phiNP[32 * q:32 * (q + 1), h * 512:(h + 1) * 512],
                    tile_position=(32 * q, 0), start=True, stop=True)
                dst = ost[:, m * 512:(m + 1) * 512]
                if m % 2 == 1:
                    nc.scalar.copy(dst, op[:])
                else:
                    nc.vector.tensor_copy(dst, op[:])
                if m + 1 in bounds:
                    nc.sync.dma_start(
                        out_ext[:, (g * 8 + lo) * 512:(g * 8 + m + 1) * 512],
                        ost[:, lo * 512:(m + 1) * 512])
                    lo = m + 1

        # PE order: G1_0, REP_0, G1_1, G2_0, REP_1, G1_2, G2_1, REP_2,
        # G1_3, G2_2, REP_3, G2_3 -- each REP matmul slots before the next
        # GEMM1 so the epilogue chain overlaps it, while the PE stays
        # busy on the next group's GEMM1 during each chain
        gemm1(0)
        epilogue_pre(0)
        gemm1(1)
        epilogue(0)
        gemm2(0)
        epilogue_pre(1)
        gemm1(2)
        epilogue(1)
        gemm2(1)
        epilogue_pre(2)
        gemm1(3)
        epilogue(2)
        gemm2(2)
        epilogue_pre(3)
        epilogue(3)
        gemm2(3)

    nc.compile()
    return nc


_CACHE = {}


def _get_nc():
    if "nc" not in _CACHE:
        _CACHE["nc"] = build_bass()
    return _CACHE["nc"]


def _host_inputs(W, b, freqs, afreqs):
    """Build the shared + per-core input arrays."""
    import ml_dtypes
    bf16 = ml_dtypes.bfloat16
    fp8 = ml_dtypes.float8_e3m4

    t, hw = _host_constants()

    phin = np.sin(np.outer(t, freqs)).astype(np.float32)     # [32, 4096]
    phinp = np.ascontiguousarray(
        phin.reshape(NQ, 4, D // 4).transpose(1, 0, 2).reshape(P, D // 4)
    ).astype(bf16)
    phit = np.ascontiguousarray(np.clip(
        phin.T.reshape(KT, P, NQ).transpose(1, 0, 2).reshape(P, KT * NQ)
        * 8.0, -14.0, 14.0)).astype(fp8)
    shared = {"phit": phit, "phinp": phinp}
    in_maps = []
    for i in range(8):
        sl = slice(i * J, (i + 1) * J)
        Wc = np.clip(W[:, sl] * WSCALE, -14.0, 14.0).astype(fp8)
        wpack = np.ascontiguousarray(
            Wc.reshape(KT, P, G, JG).transpose(1, 2, 0, 3).reshape(P, G * KT * JG))
        bpack = np.zeros((NQ, J + P), np.float32)
        bpack[0, 0:J] = b[sl] * WSCALE * 8.0
        bpack[np.arange(P) % NQ, J + np.arange(P)] = 1.0
        hwcn = -hw[:, None] * np.cos(np.outer(t, afreqs[sl]))
        m = dict(shared)
        m["w"] = wpack
        m["bpack"] = bpack.astype(bf16)
        m["hwcn"] = np.ascontiguousarray(np.tile(hwcn, (4, 1))).astype(bf16)
        in_maps.append(m)
    return in_maps


# i-chunk for GEMM2 issue index m: q = m%4 (row group / i-quarter)
_IC_OF_M = np.array([2 * (m % 4) + m // 4 for m in range(8)])


def _unpack_out(res_i):
    """[P, (g*8 + m)*512 + ii] packed out^T -> [D, J] float32."""
    x = res_i.reshape(P, G, 8, 512)        # [p, g, m, ii]
    x = x[:, :, np.argsort(_IC_OF_M), :]   # [p, g, ic, ii]
    outT = x.transpose(1, 0, 2, 3).reshape(J, D)   # [j, i]
    return np.ascontiguousarray(outT.T).astype(np.float32)


def kernel(W, b, freqs, afreqs):
    from concourse.bass_utils import run_bass_kernel_spmd

    W = np.asarray(W, dtype=np.float32)
    b = np.asarray(b, dtype=np.float32)
    freqs = np.asarray(freqs, dtype=np.float32)
    afreqs = np.asarray(afreqs, dtype=np.float32)

    nc = _get_nc()
    in_maps = _host_inputs(W, b, freqs, afreqs)
    res = run_bass_kernel_spmd(nc, in_maps, core_ids=list(range(8)))
    return np.concatenate(
        [_unpack_out(np.asarray(res.results[i]["out"])) for i in range(8)],
        axis=1)
